# revision 22
# baseline (speedup 1.0000x reference)
"""4-layer GAT + BN + ReLU + linear head on 8 Trainium2 NeuronCores.

Self-contained: takes FULL inputs (as produced by the problem's setup_inputs),
returns the FULL [50000, 32] float32 output.

Strategy (memory-regime, node-sharded):
- Nodes sharded contiguously across 8 cores (6250 each). Per layer:
    M:  h_ext_shard = zT_shard @ W_ext   (W_ext = [W | W@A_s | W@A_d | pad] -> 320 cols)
    AG: AllGather h_ext -> every core holds h_ext for all N nodes
    E:  per 128-dst-node tile, edges (dst-sorted, host-prepped) are processed in
        128-edge chunks: dma_gather fetches h_ext[src] rows (1280B each) into
        SBUF, a one-hot matrix built from iota==dstlocal drives both the
        al_d permutation (PE transpose + tiny matmul) and the segment
        sum (scatter matmul).  Softmax uses no max-subtraction (attention
        logits are bounded; exp is overflow-safe), normalization happens
        after aggregation: y = (sum ex*h) / (sum ex).
    R:  BN statistics (col sums / sumsq via ones-matmul) AllReduduced (2KB)
    BN: y tiles transposed (PE), BN+ReLU fused into one ScalarE activation
        (relu(S*y + B)) producing zT for the next layer's matmul.
- GATConv bias b is dropped: BN(y + b) == BN(y) exactly.
- int16 gather indices: sources split per tile into < 32768 and >= 32768
  groups, gathered from offset base APs. Padding uses idx=0 (valid row) and
  dstlocal=200 (matches no one-hot column -> contributes exactly zero).
- SPMD: one NEFF for all 8 cores. All per-core variation lives in input
  tensors; per-tile group sizes are padded to the max over cores.
"""

import os

import numpy as np

import concourse.bass as bass
import concourse.bacc as bacc
import concourse.mybir as mybir
import concourse.tile as tile
from concourse import bass_utils, library_config
from concourse.masks import make_identity

dt = mybir.dt
ALU = mybir.AluOpType
AFT = mybir.ActivationFunctionType

N, E, H, C = 50000, 800000, 4, 64
NCORES, SHARD, P = 8, 6250, 128
NTILE = (SHARD + P - 1) // P  # 49
HI = 32768
F = H * C  # 256
ROW = 320  # h_ext row: [h(256) | al_s(4) | al_d(4) | pad] (1280 B, %256==0)
EPS = 1e-5

LAST_RESULTS = None  # BassKernelResults of the most recent run (for test.py)
_CACHE: dict = {}


def _cnt(t):
    return min(P, SHARD - t * P)


# --------------------------------------------------------------------------
# host-side graph preprocessing
# --------------------------------------------------------------------------

def _prep_graph(ei):
    """Build per-core packed gather-index / dstlocal arrays + shared meta.

    Returns (idx_arrs[NCORES] int16 [128, ICOLS], dfl_arrs[NCORES] f32
    [128, CCOLS], meta) where meta is a list over tiles of per-group
    descriptors (icol, npad, is_hi, cbase, nchunk) common to all cores.
    """
    src = np.asarray(ei[0], np.int64)
    dst = np.asarray(ei[1], np.int64)
    sl = np.arange(N, dtype=np.int64)
    src = np.concatenate([src, sl])
    dst = np.concatenate([dst, sl])
    order = np.argsort(dst, kind="stable")
    src, dst = src[order], dst[order]

    bounds = [
        min(r * SHARD + t * P, (r + 1) * SHARD)
        for r in range(NCORES)
        for t in range(NTILE)
    ] + [N]
    starts = np.searchsorted(dst, np.asarray(bounds))

    raw = [[None] * NTILE for _ in range(NCORES)]
    for r in range(NCORES):
        for t in range(NTILE):
            g = r * NTILE + t
            a, b = starts[g], starts[g + 1]
            s_t = src[a:b]
            dl = dst[a:b] - (r * SHARD + t * P)
            lo = s_t < HI
            raw[r][t] = (
                (s_t[lo], dl[lo]),
                (s_t[~lo] - HI, dl[~lo]),
            )

    npad = np.zeros((NTILE, 2), np.int64)
    for t in range(NTILE):
        for g in range(2):
            mx = max(len(raw[r][t][g][0]) for r in range(NCORES))
            npad[t, g] = ((max(mx, 1) + P - 1) // P) * P

    meta = []
    icol = ccol = 0
    for t in range(NTILE):
        groups = []
        cbase = 0
        for g in range(2):
            n = int(npad[t, g])
            nchunk = n // P
            groups.append((icol, n, g == 1, cbase, nchunk))
            icol += n // 16
            cbase += nchunk
        ccol += cbase
        meta.append(groups)
    ICOLS, CCOLS = icol, ccol

    idx_arrs, dfl_arrs = [], []
    for r in range(NCORES):
        idx = np.zeros((16, ICOLS), np.int16)
        dfl = np.full((P, CCOLS), 200.0, np.float32)
        ccur = 0
        for t in range(NTILE):
            for g in range(2):
                ic, n, _, cb, nchunk = meta[t][g]
                s_t, dl = raw[r][t][g]
                ii = np.zeros(n, np.int16)
                ii[: len(s_t)] = s_t.astype(np.int16)
                idx[:, ic : ic + n // 16] = ii.reshape(n // 16, 16).T
                dd = np.full(n, 200.0, np.float32)
                dd[: len(dl)] = dl.astype(np.float32)
                dfl[:, ccur + cb : ccur + cb + nchunk] = dd.reshape(nchunk, P).T
            ccur += meta[t][0][4] + meta[t][1][4]
        # replicate the 16-partition wrap to 128 partitions (8 q7 cores)
        idx_arrs.append(np.tile(idx, (8, 1)).astype(np.int16))
        dfl_arrs.append(dfl)
    return idx_arrs, dfl_arrs, meta, ICOLS, CCOLS


def _make_wext(W, a_s, a_d):
    ind = W.shape[0]
    A_s = np.zeros((F, H), np.float32)
    A_d = np.zeros((F, H), np.float32)
    for h in range(H):
        A_s[h * C : (h + 1) * C, h] = a_s[h]
        A_d[h * C : (h + 1) * C, h] = a_d[h]
    Wx = np.zeros((ind, ROW), np.float32)
    Wx[:, :F] = W
    Wx[:, 256:260] = W @ A_s
    Wx[:, 260:264] = W @ A_d
    return np.ascontiguousarray(Wx)


# --------------------------------------------------------------------------
# device kernel builder
# --------------------------------------------------------------------------

def _build(meta_sp, meta_te, ICOLS_sp, CCOLS_sp, ICOLS_te, CCOLS_te):
    ICOLS = max(ICOLS_sp, ICOLS_te)
    CCOLS = max(CCOLS_sp, CCOLS_te)
    S_MAX = max(
        m[0][4] + m[1][4] for m in (list(meta_sp) + list(meta_te))
    )

    nc = bacc.Bacc(
        "TRN2", target_bir_lowering=False, debug=False, num_devices=NCORES
    )

    f32, i16, i32 = dt.float32, dt.int16, dt.int32

    xT_t = nc.dram_tensor("xT", [P, SHARD], f32, kind="ExternalInput")
    idx_t = {
        g: nc.dram_tensor(f"idx_{g}", [P, ICOLS], i16, kind="ExternalInput")
        for g in ("sp", "te")
    }
    dfl_t = {
        g: nc.dram_tensor(f"dfl_{g}", [P, CCOLS], f32, kind="ExternalInput")
        for g in ("sp", "te")
    }
    wext_t, gT_t, beT_t = {}, {}, {}
    for l in range(1, 5):
        kin = 128 if l == 1 else 256
        wext_t[l] = nc.dram_tensor(f"wext{l}", [kin, ROW], f32, kind="ExternalInput")
        gT_t[l] = nc.dram_tensor(f"gT{l}", [P, 2], f32, kind="ExternalInput")
        beT_t[l] = nc.dram_tensor(f"beT{l}", [P, 2], f32, kind="ExternalInput")
    wl_t = nc.dram_tensor("wl", [F, 32], f32, kind="ExternalInput")
    bl_t = nc.dram_tensor("bl2", [1, 32], f32, kind="ExternalInput")
    out_t = nc.dram_tensor("out", [SHARD, 32], f32, kind="ExternalOutput")
    debug = bool(int(os.environ.get("GAT_DEBUG", "0")))
    if debug:
        dbg_hx_t = nc.dram_tensor("dbg_hx", [N, ROW], f32, kind="ExternalOutput")
        dbg_y_t = nc.dram_tensor("dbg_y", [SHARD, F], f32, kind="ExternalOutput")
        dbg_z_t = nc.dram_tensor("dbg_z", [P, 2 * SHARD], f32, kind="ExternalOutput")
        dbg_sb_t = nc.dram_tensor("dbg_sb", [P, 8], f32, kind="ExternalOutput")

    RG = [list(range(NCORES))]

    with tile.TileContext(nc) as tc:
        with (
            tc.tile_pool(name="dram", bufs=1, space="DRAM") as dpool,
            tc.tile_pool(name="const", bufs=1) as cpool,
            tc.tile_pool(name="zpool", bufs=1) as zpool,
            tc.tile_pool(name="gidx", bufs=1) as gipool,
            tc.tile_pool(name="gpool", bufs=2) as gpool,
            tc.tile_pool(name="g2pool", bufs=2) as g2pool,
            tc.tile_pool(name="small", bufs=3) as spool,
            tc.tile_pool(name="wpool", bufs=1) as wpool,
            tc.tile_pool(name="hpool", bufs=3) as hpool,
            tc.tile_pool(name="ypool", bufs=2) as ypool,
            tc.tile_pool(name="pbig", bufs=2, space="PSUM") as pbig,
            tc.tile_pool(name="pt", bufs=2, space="PSUM") as ptp,
            tc.tile_pool(name="ps", bufs=2, space="PSUM") as psp,
            tc.tile_pool(name="pstat", bufs=2, space="PSUM") as pstatp,
        ):
            # internal DRAM (tracked pool tiles). Shared tensors allow only a
            # single writer instruction -> one per layer.
            ag_in = dpool.tile([SHARD, ROW], f32, name="ag_in")
            ag_outs = [
                dpool.tile([N, ROW], f32, addr_space="Shared", name=f"ag_out{l}")
                for l in range(4)
            ]
            y_dram = dpool.tile([SHARD, F], f32, name="y_dram")
            ar_in = dpool.tile([P, 4], f32, name="ar_in")
            ar_outs = [
                dpool.tile([P, 4], f32, addr_space="Shared", name=f"ar_out{l}")
                for l in range(4)
            ]

            # ---- constants
            ident = cpool.tile([P, P], f32, name="ident")
            make_identity(nc, ident[:])
            iota32 = cpool.tile([P, P], i32, name="iota32")
            nc.gpsimd.iota(iota32[:], pattern=[[1, P]], base=0, channel_multiplier=0)
            iotaF = cpool.tile([P, P], f32, name="iotaF")
            nc.vector.tensor_copy(iotaF[:], iota32[:])
            ones_col = cpool.tile([P, 1], f32, name="ones_col")
            nc.vector.memset(ones_col[:], 1.0)
            ones_row = cpool.tile([1, P], f32, name="ones_row")
            nc.vector.memset(ones_row[:], 1.0)

            # persistent activation (transposed): [128, 2*SHARD]
            zT = zpool.tile([P, 2 * SHARD], f32, name="zT", tag="zT")
            nc.sync.dma_start(zT[:, 0:SHARD], xT_t[:])

            wl_sb = cpool.tile([P, 2, 32], f32, name="wl_sb")
            for k in range(2):
                nc.sync.dma_start(wl_sb[:, k, :], wl_t[k * P : (k + 1) * P, :])
            bl_sb = cpool.tile([1, 32], f32, name="bl_sb")
            nc.sync.dma_start(bl_sb[:], bl_t[:])

            n_layers = int(os.environ.get("GAT_NLAYERS", "4"))
            for l in range(1, n_layers + 1):
                graph = "sp" if l <= 2 else "te"
                meta = meta_sp if l <= 2 else meta_te
                KC = 1 if l == 1 else 2

                # ---- per-layer loads
                if l in (1, 3):
                    idx_sb = gipool.tile([P, ICOLS], i16, name=f"idx{l}", tag="idx")
                    nc.sync.dma_start(idx_sb[:], idx_t[graph][:])
                    dfl_sb = gipool.tile([P, CCOLS], f32, name=f"dfl{l}", tag="dfl")
                    nc.sync.dma_start(dfl_sb[:], dfl_t[graph][:])
                wsb = wpool.tile([P, 2, ROW], f32, name=f"w{l}", tag="w")
                for k in range(KC):
                    nc.sync.dma_start(
                        wsb[:, k, :], wext_t[l][k * P : k * P + P, :]
                    )
                gT_sb = wpool.tile([P, 2], f32, name=f"g{l}", tag="gT")
                nc.sync.dma_start(gT_sb[:], gT_t[l][:])
                beT_sb = wpool.tile([P, 2], f32, name=f"be{l}", tag="beT")
                nc.sync.dma_start(beT_sb[:], beT_t[l][:])

                al_d_all = wpool.tile([P, NTILE * 4], f32, name=f"ald{l}", tag="ald")
                nc.vector.memset(al_d_all[:], 0.0)

                # ---- phase M: h_ext_shard = zT.T @ W_ext
                for t in range(NTILE):
                    cnt = _cnt(t)
                    ph = pbig.tile([P, ROW], f32, name=f"ph{l}_{t}", tag="pbig")
                    for k in range(KC):
                        nc.tensor.matmul(
                            ph[:cnt, :],
                            zT[:, k * SHARD + t * P : k * SHARD + t * P + cnt],
                            wsb[:, k, :],
                            start=(k == 0),
                            stop=(k == KC - 1),
                        )
                    hsb = hpool.tile([P, ROW], f32, name=f"h{l}_{t}", tag="hsb")
                    nc.vector.tensor_copy(hsb[:cnt, :], ph[:cnt, :])
                    nc.vector.tensor_copy(
                        al_d_all[:cnt, t * 4 : t * 4 + 4], ph[:cnt, 260:264]
                    )
                    nc.sync.dma_start(
                        ag_in[t * P : t * P + cnt, :], hsb[:cnt, :]
                    )

                # ---- AllGather h_ext
                ag_out = ag_outs[l - 1]
                nc.gpsimd.collective_compute(
                    "AllGather",
                    ALU.bypass,
                    replica_groups=RG,
                    ins=[ag_in[:]],
                    outs=[ag_out[:]],
                )
                if debug and l == 1:
                    for db in range(N // P + 1):
                        dcnt = min(P, N - db * P)
                        dbt = hpool.tile([P, ROW], f32, name=f"dbgh{db}", tag="hsb")
                        nc.sync.dma_start(
                            dbt[:dcnt, :], ag_out[db * P : db * P + dcnt, :]
                        )
                        nc.sync.dma_start(
                            dbg_hx_t[db * P : db * P + dcnt, :], dbt[:dcnt, :]
                        )

                # ---- phase E: per-tile attention + aggregation
                stats_sb = wpool.tile([P, 4], f32, name=f"stats{l}", tag="stats")
                nc.vector.memset(stats_sb[:], 0.0)
                for t in range(NTILE):
                    cnt = _cnt(t)
                    S_t = meta[t][0][4] + meta[t][1][4]
                    ccur = sum(m[0][4] + m[1][4] for m in meta[:t])

                    G = gpool.tile([P, S_MAX * ROW], f32, name=f"G{l}_{t}", tag="G")
                    Gv = G[:].rearrange("p (s r) -> p s r", s=S_MAX)
                    for ic, n, is_hi, cb, nchunk in meta[t]:
                        src_ap = ag_out[HI:N, :] if is_hi else ag_out[0:HI, :]
                        off = 0
                        while off < n:  # q7 dma_gather caps at 1024 idxs
                            nn = min(1024, n - off)
                            nc.gpsimd.dma_gather(
                                Gv[:, cb + off // P : cb + (off + nn) // P, :],
                                src_ap,
                                idx_sb[:, ic + off // 16 : ic + (off + nn) // 16],
                                num_idxs=nn,
                                num_idxs_reg=nn,
                                elem_size=ROW,
                                elem_step=ROW,
                            )
                            off += nn

                    G2 = g2pool.tile(
                        [P, S_MAX * 260], f32, name=f"G2{l}_{t}", tag="G2"
                    )
                    out_ext = pbig.tile(
                        [P, 260], f32, name=f"oe{l}_{t}", tag="pbig"
                    )
                    for c in range(S_t):
                        dcol = dfl_sb[:, ccur + c : ccur + c + 1]
                        onehot = spool.tile([P, P], f32, name=f"oh{l}_{t}_{c}", tag="oh")
                        nc.gpsimd.tensor_scalar(
                            onehot[:], iotaF[:], dcol, None, ALU.is_equal
                        )
                        ohT_ps = ptp.tile([P, P], f32, name=f"ot{l}_{t}_{c}", tag="pt")
                        nc.tensor.transpose(ohT_ps[:], onehot[:], ident[:])
                        ohT = spool.tile([P, P], f32, name=f"oT{l}_{t}_{c}", tag="ohT")
                        nc.scalar.copy(ohT[:], ohT_ps[:])
                        alde = psp.tile([P, 4], f32, name=f"ae{l}_{t}_{c}", tag="ps")
                        nc.tensor.matmul(
                            alde[:],
                            ohT[:],
                            al_d_all[:, t * 4 : t * 4 + 4],
                            start=True,
                            stop=True,
                        )
                        tv = spool.tile([P, 4], f32, name=f"tv{l}_{t}_{c}", tag="tv")
                        nc.vector.tensor_tensor(
                            out=tv[:],
                            in0=alde[:],
                            in1=G[:, c * ROW + 256 : c * ROW + 260],
                            op=ALU.add,
                        )
                        # exp(leaky_relu(t)) = max(exp(t), exp(0.2 t))
                        e1 = spool.tile([P, 4], f32, name=f"e1{l}_{t}_{c}", tag="e1")
                        nc.scalar.activation(e1[:], tv[:], AFT.Exp)
                        e2 = spool.tile([P, 4], f32, name=f"e2{l}_{t}_{c}", tag="e2")
                        nc.scalar.activation(e2[:], tv[:], AFT.Exp, scale=0.2)
                        exs = G2[:, c * 260 + 256 : c * 260 + 260]
                        nc.vector.tensor_tensor(
                            out=exs, in0=e1[:], in1=e2[:], op=ALU.max
                        )
                        nc.vector.tensor_tensor(
                            out=G2[:, c * 260 : c * 260 + 256].rearrange(
                                "p (h c) -> p h c", h=H
                            ),
                            in0=G[:, c * ROW : c * ROW + 256].rearrange(
                                "p (h c) -> p h c", h=H
                            ),
                            in1=exs.to_broadcast([P, H, C]),
                            op=ALU.mult,
                        )
                        nc.tensor.matmul(
                            out_ext[:],
                            onehot[:],
                            G2[:, c * 260 : (c + 1) * 260],
                            start=(c == 0),
                            stop=(c == S_t - 1),
                        )

                    # tile epilogue: normalize, stats, store y
                    rs = spool.tile([P, 4], f32, name=f"rs{l}_{t}", tag="rs")
                    nc.vector.tensor_scalar(
                        rs[:], out_ext[:, 256:260], 1e-16, None, ALU.add
                    )
                    nc.vector.reciprocal(rs[:], rs[:])
                    y_sb = ypool.tile([P, F], f32, name=f"y{l}_{t}", tag="y")
                    nc.vector.tensor_tensor(
                        out=y_sb[:].rearrange("p (h c) -> p h c", h=H),
                        in0=out_ext[:, 0:F].rearrange("p (h c) -> p h c", h=H),
                        in1=rs[:].to_broadcast([P, H, C]),
                        op=ALU.mult,
                    )
                    ysq = ypool.tile([P, F], f32, name=f"yq{l}_{t}", tag="ysq")
                    nc.scalar.activation(ysq[:cnt, :], y_sb[:cnt, :], AFT.Square)
                    stp = pstatp.tile([P, 4], f32, name=f"stp{l}_{t}", tag="pstat")
                    for j, ssrc in enumerate(
                        (y_sb[:, 0:128], y_sb[:, 128:256], ysq[:, 0:128], ysq[:, 128:256])
                    ):
                        nc.tensor.matmul(
                            stp[:, j : j + 1],
                            ssrc[:cnt, :],
                            ones_col[:cnt, :],
                            start=True,
                            stop=True,
                        )
                    nc.vector.tensor_tensor(
                        out=stats_sb[:], in0=stats_sb[:], in1=stp[:], op=ALU.add
                    )
                    nc.sync.dma_start(
                        y_dram[t * P : t * P + cnt, :], y_sb[:cnt, :]
                    )

                if debug and l == 1:
                    for db in range(NTILE):
                        dcnt = _cnt(db)
                        dbt2 = ypool.tile([P, F], f32, name=f"dbgy{db}", tag="y2")
                        nc.sync.dma_start(
                            dbt2[:dcnt, :], y_dram[db * P : db * P + dcnt, :]
                        )
                        nc.sync.dma_start(
                            dbg_y_t[db * P : db * P + dcnt, :], dbt2[:dcnt, :]
                        )

                # ---- phase R: AllReduce BN stats, compute S/B
                nc.sync.dma_start(ar_in[:], stats_sb[:])
                ar_out = ar_outs[l - 1]
                nc.gpsimd.collective_compute(
                    "AllReduce",
                    ALU.add,
                    replica_groups=RG,
                    ins=[ar_in[:]],
                    outs=[ar_out[:]],
                )
                arf = spool.tile([P, 4], f32, name=f"arf{l}", tag="arf")
                nc.sync.dma_start(arf[:], ar_out[:])
                mean = spool.tile([P, 2], f32, name=f"mean{l}", tag="mean")
                nc.vector.tensor_scalar(
                    mean[:], arf[:, 0:2], 1.0 / N, None, ALU.mult
                )
                var = spool.tile([P, 2], f32, name=f"var{l}", tag="var")
                nc.vector.tensor_scalar(var[:], arf[:, 2:4], 1.0 / N, None, ALU.mult)
                msq = spool.tile([P, 2], f32, name=f"msq{l}", tag="msq")
                nc.vector.tensor_tensor(out=msq[:], in0=mean[:], in1=mean[:], op=ALU.mult)
                nc.vector.tensor_tensor(out=var[:], in0=var[:], in1=msq[:], op=ALU.subtract)
                nc.vector.tensor_scalar(var[:], var[:], EPS, None, ALU.add)
                sd = spool.tile([P, 2], f32, name=f"sd{l}", tag="sd")
                nc.scalar.activation(sd[:], var[:], AFT.Sqrt)
                nc.vector.reciprocal(sd[:], sd[:])
                Sb = wpool.tile([P, 2], f32, name=f"S{l}", tag="Sb")
                nc.vector.tensor_tensor(out=Sb[:], in0=gT_sb[:], in1=sd[:], op=ALU.mult)
                Bb = wpool.tile([P, 2], f32, name=f"B{l}", tag="Bb")
                nc.vector.tensor_tensor(out=Bb[:], in0=mean[:], in1=Sb[:], op=ALU.mult)
                nc.vector.tensor_tensor(out=Bb[:], in0=beT_sb[:], in1=Bb[:], op=ALU.subtract)

                # ---- phase BN: zT = relu(S*y + B) on transposed tiles
                for t in range(NTILE):
                    cnt = _cnt(t)
                    y2 = ypool.tile([P, F], f32, name=f"y2{l}_{t}", tag="y2")
                    nc.sync.dma_start(y2[:cnt, :], y_dram[t * P : t * P + cnt, :])
                    for k in range(2):
                        yT = ptp.tile([P, P], f32, name=f"yt{l}_{t}_{k}", tag="pt")
                        nc.tensor.transpose(
                            yT[:, :cnt],
                            y2[:cnt, k * P : (k + 1) * P],
                            ident[:cnt, :cnt],
                        )
                        nc.scalar.activation(
                            zT[:, k * SHARD + t * P : k * SHARD + t * P + cnt],
                            yT[:, :cnt],
                            AFT.Relu,
                            bias=Bb[:, k : k + 1],
                            scale=Sb[:, k : k + 1],
                        )

            if debug:
                nc.sync.dma_start(dbg_z_t[:], zT[:])
                nc.sync.dma_start(dbg_sb_t[:, 0:2], Sb[:])
                nc.sync.dma_start(dbg_sb_t[:, 2:4], Bb[:])
                nc.sync.dma_start(dbg_sb_t[:, 4:8], arf[:])

            # ---- final projection: out = z4 @ Wl + bl
            for t in range(NTILE):
                cnt = _cnt(t)
                po = psp.tile([P, 32], f32, name=f"po{t}", tag="ps")
                for k in range(2):
                    nc.tensor.matmul(
                        po[:cnt, :],
                        zT[:, k * SHARD + t * P : k * SHARD + t * P + cnt],
                        wl_sb[:, k, :],
                        start=(k == 0),
                        stop=False,
                    )
                nc.tensor.matmul(
                    po[:cnt, :],
                    ones_row[:, :cnt],
                    bl_sb[:],
                    start=False,
                    stop=True,
                )
                osb = hpool.tile([P, 32], f32, name=f"o{t}", tag="osb")
                nc.vector.tensor_copy(osb[:cnt, :], po[:cnt, :])
                nc.sync.dma_start(out_t[t * P : t * P + cnt, :], osb[:cnt, :])

    nc.compile()
    return nc


# --------------------------------------------------------------------------
# entry point
# --------------------------------------------------------------------------

def kernel(**inputs) -> np.ndarray:
    global LAST_RESULTS

    x = np.ascontiguousarray(np.asarray(inputs["x"], np.float32))
    key = (
        int(np.asarray(inputs["edge_index_spatial"]).sum()),
        int(np.asarray(inputs["edge_index_temporal"]).sum()),
    )
    if key in _CACHE:
        nc, idx_sp, dfl_sp, idx_te, dfl_te, ICOLS = _CACHE[key]
    else:
        idx_sp, dfl_sp, meta_sp, IC_sp, CC_sp = _prep_graph(
            inputs["edge_index_spatial"]
        )
        idx_te, dfl_te, meta_te, IC_te, CC_te = _prep_graph(
            inputs["edge_index_temporal"]
        )
        ICOLS = max(IC_sp, IC_te)
        CCOLS = max(CC_sp, CC_te)
        # pad per-core arrays to the common max cols
        idx_sp = [_pad2(a, ICOLS) for a in idx_sp]
        idx_te = [_pad2(a, ICOLS) for a in idx_te]
        dfl_sp = [_pad2(a, CCOLS) for a in dfl_sp]
        dfl_te = [_pad2(a, CCOLS) for a in dfl_te]
        nc = _build(meta_sp, meta_te, IC_sp, CC_sp, IC_te, CC_te)
        _CACHE[key] = (nc, idx_sp, dfl_sp, idx_te, dfl_te, ICOLS)

    in_maps = []
    for r in range(NCORES):
        m = {
            "xT": np.ascontiguousarray(x[r * SHARD : (r + 1) * SHARD, :].T),
            "idx_sp": idx_sp[r],
            "dfl_sp": dfl_sp[r],
            "idx_te": idx_te[r],
            "dfl_te": dfl_te[r],
            "wl": np.ascontiguousarray(np.asarray(inputs["Wl"], np.float32)),
            "bl2": np.ascontiguousarray(
                np.asarray(inputs["bl"], np.float32).reshape(1, 32)
            ),
        }
        for l in range(1, 5):
            m[f"wext{l}"] = _make_wext(
                np.asarray(inputs[f"W{l}"], np.float32),
                np.asarray(inputs[f"as{l}"], np.float32),
                np.asarray(inputs[f"ad{l}"], np.float32),
            )
            m[f"gT{l}"] = np.ascontiguousarray(
                np.asarray(inputs[f"g{l}"], np.float32).reshape(2, P).T
            )
            m[f"beT{l}"] = np.ascontiguousarray(
                np.asarray(inputs[f"be{l}"], np.float32).reshape(2, P).T
            )
        in_maps.append(m)

    trace = bool(int(os.environ.get("GAT_TRACE", "0")))
    try:
        res = bass_utils.run_bass_kernel_spmd(
            nc, in_maps, core_ids=list(range(NCORES)), trace=trace
        )
    except ModuleNotFoundError:
        res = bass_utils.run_bass_kernel_spmd(
            nc, in_maps, core_ids=list(range(NCORES)), trace=False
        )
    LAST_RESULTS = res
    return np.concatenate([res.results[r]["out"] for r in range(NCORES)], axis=0)


def _pad2(a, cols):
    if a.shape[1] == cols:
        return np.ascontiguousarray(a)
    out = np.zeros((a.shape[0], cols), a.dtype)
    out[:, : a.shape[1]] = a
    return out
